# revision 20
# baseline (speedup 1.0000x reference)
"""Trainium2 Bass kernel for nn_DAGExecutor (B=8, T=4096, dag_depth=16, nodes=32).

Sharding: pure data parallel over the batch dim — batch b runs on core b.
Each core processes T=4096 positions laid out as [128 partitions x F=32].

Math restructure (numerically validated against the reference, incl. inf/NaN):
  For each position and step s, the node axis splits into
    - static nodes j<16 (initial V values, never overwritten, always finite):
      P[s] = sum_j O[s,j]*log(Vm0[j]),  Q[s] = sum_j O[s,j]*Vs0[j]*Vm0[j],
      SP0[s] = prod_j (Vs0[j]*2|O[s,j]|+1)   -- all precomputed in bulk ops
    - dynamic nodes j=16+s' (s'<s), written by the recurrence (may be inf):
      consumed per step with the reference's exact form
      mixed = (1-G)*lm + G*sg to reproduce inf/NaN propagation.
  R[s] = (1-G[s])*P[s] + G[s]*Q[s] + sum_{s'<s} Odyn[s,s']*mixed[s']
  sign_prod[s] = SP0[s] * prod_{s'<s} (vs[s']*2|Odyn[s,s']|+1)

tanh(x) is computed as sign(x)*(1-e)/(1+e), e=exp(-2|x|), so that every
activation (Exp/Ln/Sign) lives in the single ACT table set
natural_log_exp_and_others -- avoids per-step table reloads.

Host-side prep is layout-only: slicing, transposing, triangular packing.
The masked region of O (nodes >= 16+s at step s) is never shipped.
"""
import numpy as np
from contextlib import ExitStack

import concourse.bass as bass
import concourse.bacc as bacc
import concourse.mybir as mybir
from concourse.tile import TileContext
from concourse.bass_utils import run_bass_kernel_spmd

f32 = np.float32
DT = mybir.dt.float32
Alu = mybir.AluOpType
Act = mybir.ActivationFunctionType
AX = mybir.AxisListType

B, T, S, NODES = 8, 4096, 16, 32

_ACT_SET = "natural_log_exp_and_others"  # contains Exp, Ln, Sign


class _OneActSetBacc(bacc.Bacc):
    """Bacc whose act-table-load pass may only pick one function set.

    The stock pass greedily picks the first set containing each activation's
    function, which alternates exp_and_others / natural_log across the step
    loop (one ~2.7us table DMA per switch). Every function this kernel uses
    (Exp, Ln, Sign) lives in natural_log_exp_and_others, so restricting the
    candidate list to that set yields a single hoisted load. Indices into
    act_info.json's act_func_sets are preserved (walrus remaps by index).
    """

    def insert_act_table_loads(self):
        from concourse.hw_specs import get_activation_tables
        import bass_rust as _bass_rust
        has_activation = any(
            isinstance(i, mybir.InstActivation)
            for b in self.main_func.blocks for i in b.instructions)
        if not has_activation:
            return
        tables = [
            (name, (fns if name == _ACT_SET else set()))
            for name, fns in get_activation_tables(self.m.arch).items()
        ]
        _bass_rust.insert_act_table_loads(self, tables)
P = 128
TRI = S * (S - 1) // 2  # 120
OFF = [s * (s - 1) // 2 for s in range(S + 1)]
LOG_LIM = 100.0
NTANH = -2.0e4  # -2/TEMP
FLT_MAX = float(np.finfo(np.float32).max)
LN2 = float(np.log(np.float64(2.0)))


def build_program(F, n_devices=8):
    """Build + compile the per-core program for F positions per partition."""
    nc = _OneActSetBacc("TRN2", target_bir_lowering=False, debug=False,
                        num_devices=n_devices)
    vm0_d = nc.dram_tensor("vm0", [P, F * 16], DT, kind="ExternalInput")
    vs0_d = nc.dram_tensor("vs0", [P, F * 16], DT, kind="ExternalInput")
    g_d = nc.dram_tensor("g", [P, S * F], DT, kind="ExternalInput")
    ost_d = nc.dram_tensor("ost", [P, S * F * 16], DT, kind="ExternalInput")
    ody_d = nc.dram_tensor("ody", [P, TRI * F], DT, kind="ExternalInput")
    out_d = nc.dram_tensor("out", [P, F], DT, kind="ExternalOutput")

    SCHUNK = 8  # s-values per static chunk
    NCH = S // SCHUNK

    with TileContext(nc) as tc, ExitStack() as ctx:
        state = ctx.enter_context(tc.tile_pool(name="state", bufs=1))
        ost_pool = ctx.enter_context(tc.tile_pool(name="ost", bufs=2))
        big = ctx.enter_context(tc.tile_pool(name="big", bufs=2))
        sm = ctx.enter_context(tc.tile_pool(name="sm", bufs=3))

        # ---- persistent state tiles ----
        vm0 = state.tile([P, F * 16], DT, tag="vm0")
        vs0 = state.tile([P, F * 16], DT, tag="vs0")
        gt = state.tile([P, S * F], DT, tag="gt")
        ct = state.tile([P, S * F], DT, tag="ct")
        lm0 = state.tile([P, F * 16], DT, tag="lm0")
        sg0 = state.tile([P, F * 16], DT, tag="sg0")
        Pt = state.tile([P, S * F], DT, tag="Pt")
        Qt = state.tile([P, S * F], DT, tag="Qt")
        SPt = state.tile([P, S * F], DT, tag="SPt")
        lmh = state.tile([P, S * F], DT, tag="lmh")
        sgh = state.tile([P, S * F], DT, tag="sgh")
        vsh = state.tile([P, S * F], DT, tag="vsh")
        ody = state.tile([P, TRI * F], DT, tag="ody")
        odyA = state.tile([P, TRI * F], DT, tag="odyA")
        outt = state.tile([P, F], DT, tag="outt")

        nc.sync.dma_start(vm0[:], vm0_d.ap())
        nc.sync.dma_start(vs0[:], vs0_d.ap())
        nc.sync.dma_start(gt[:], g_d.ap())
        nc.sync.dma_start(ody[:], ody_d.ap())
        # 2*|O_dyn| for the sign-weight products, one bulk ACT op
        nc.scalar.activation(odyA[:], ody[:], Act.Abs, scale=2.0)

        # ---- stage A ----
        nc.vector.tensor_scalar(ct[:], gt[:], -1.0, 1.0, Alu.mult, Alu.add)
        nc.scalar.activation(lm0[:], vm0[:], Act.Ln)
        nc.vector.tensor_tensor(sg0[:], vs0[:], vm0[:], Alu.mult)

        lm0b = lm0[:].rearrange("p (f j) -> p f j", f=F, j=16) \
            .unsqueeze(1).to_broadcast([P, SCHUNK, F, 16])
        sg0b = sg0[:].rearrange("p (f j) -> p f j", f=F, j=16) \
            .unsqueeze(1).to_broadcast([P, SCHUNK, F, 16])
        vs0b = vs0[:].rearrange("p (f j) -> p f j", f=F, j=16) \
            .unsqueeze(1).to_broadcast([P, SCHUNK, F, 16])

        # ---- stage B: static prefix, per s-chunk ----
        for k in range(NCH):
            ostk = ost_pool.tile([P, SCHUNK * F * 16], DT, tag="ostk")
            nc.sync.dma_start(
                ostk[:], ost_d.ap()[:, k * SCHUNK * F * 16:(k + 1) * SCHUNK * F * 16])
            o4 = ostk[:].rearrange("p (s f j) -> p s f j", s=SCHUNK, f=F, j=16)
            cols = slice(k * SCHUNK * F, (k + 1) * SCHUNK * F)

            t1 = big.tile([P, SCHUNK * F * 16], DT, tag="t1")
            t14 = t1[:].rearrange("p (s f j) -> p s f j", s=SCHUNK, f=F, j=16)
            nc.vector.tensor_tensor(t14, o4, lm0b, Alu.mult)
            nc.vector.tensor_reduce(
                Pt[:, cols].rearrange("p (s f) -> p s f", s=SCHUNK, f=F),
                t14, AX.X, Alu.add)

            t2 = big.tile([P, SCHUNK * F * 16], DT, tag="t1")
            t24 = t2[:].rearrange("p (s f j) -> p s f j", s=SCHUNK, f=F, j=16)
            # offload the second big multiply to GPSIMD (DVE is the
            # bottleneck engine; GPSIMD is otherwise idle)
            nc.gpsimd.tensor_tensor(t24, o4, sg0b, Alu.mult)
            nc.vector.tensor_reduce(
                Qt[:, cols].rearrange("p (s f) -> p s f", s=SCHUNK, f=F),
                t24, AX.X, Alu.add)

            t3 = big.tile([P, SCHUNK * F * 16], DT, tag="t1")
            t34 = t3[:].rearrange("p (s f j) -> p s f j", s=SCHUNK, f=F, j=16)
            # w = vs0 * 2|O| + 1  (abs on ACT: abs_max is reduce-only ISA;
            # multiply on GPSIMD, +1 on ACT — keeps DVE free for reduces)
            nc.scalar.activation(t34, o4, Act.Abs, scale=2.0)
            nc.gpsimd.tensor_tensor(t34, t34, vs0b, Alu.mult)
            nc.scalar.activation(t34, t34, Act.Identity, bias=1.0)
            # product over j via a binary multiply tree (reduce-mult is not
            # supported by the DVE)
            for h in (8, 4, 2):
                nc.vector.tensor_tensor(
                    t34[:, :, :, 0:h], t34[:, :, :, 0:h], t34[:, :, :, h:2 * h],
                    Alu.mult)
            nc.vector.tensor_tensor(
                SPt[:, cols].rearrange("p (s f) -> p s f", s=SCHUNK, f=F)
                .unsqueeze(3),
                t34[:, :, :, 0:1], t34[:, :, :, 1:2], Alu.mult)

        # R static part for all steps at once: Rst = c*P + g*Q
        Rst = state.tile([P, S * F], DT, tag="Rst")
        nc.vector.tensor_tensor(Rst[:], ct[:], Pt[:], Alu.mult)
        nc.vector.tensor_tensor(Qt[:], gt[:], Qt[:], Alu.mult)
        nc.vector.tensor_tensor(Rst[:], Rst[:], Qt[:], Alu.add)

        # ---- stage C: the 16-step recurrence ----
        for s in range(S):
            col = slice(s * F, (s + 1) * F)
            gs = gt[:, col]
            cs = ct[:, col]

            RS = sm.tile([P, 2 * F], DT, tag="RS")

            if s == 0:
                nc.vector.tensor_copy(RS[:, 0:F], Rst[:, col])
                nc.vector.tensor_copy(RS[:, F:2 * F], SPt[:, col])
            else:
                lmp = lmh[:, 0:s * F].rearrange("p (z f) -> p z f", z=s, f=F)
                sgp = sgh[:, 0:s * F].rearrange("p (z f) -> p z f", z=s, f=F)
                vsp = vsh[:, 0:s * F].rearrange("p (z f) -> p z f", z=s, f=F)
                cb = cs.unsqueeze(1).to_broadcast([P, s, F])
                gb = gs.unsqueeze(1).to_broadcast([P, s, F])
                od = ody[:, OFF[s] * F:(OFF[s] + s) * F] \
                    .rearrange("p (z f) -> p z f", z=s, f=F)

                m1 = sm.tile([P, 16 * F], DT, tag="m1")
                m2 = sm.tile([P, 16 * F], DT, tag="m2")
                m13 = m1[:, 0:s * F].rearrange("p (z f) -> p z f", z=s, f=F)
                m23 = m2[:, 0:s * F].rearrange("p (z f) -> p z f", z=s, f=F)
                # mixed = c*lm + g*sg  (reference's exact elementwise form)
                nc.vector.tensor_tensor(m13, lmp, cb, Alu.mult)
                nc.vector.tensor_tensor(m23, sgp, gb, Alu.mult)
                nc.vector.tensor_tensor(m13, m13, m23, Alu.add)
                nc.vector.tensor_tensor(m23, od, m13, Alu.mult)
                # reduce over s' (stride-F axis innermost)
                m23T = m2[:, 0:s * F].rearrange("p (z f) -> p f z", z=s, f=F)
                tC = sm.tile([P, F], DT, tag="tC")
                nc.vector.tensor_reduce(tC[:], m23T, AX.X, Alu.add)
                nc.vector.tensor_tensor(RS[:, 0:F], Rst[:, col], tC[:],
                                        Alu.add)
                # sign-weight dynamic product: rows s..15 padded with 1.0,
                # then a binary multiply tree over the 16 rows
                nc.gpsimd.memset(m1[:, s * F:16 * F], 1.0)
                odA = odyA[:, OFF[s] * F:(OFF[s] + s) * F] \
                    .rearrange("p (z f) -> p z f", z=s, f=F)
                nc.gpsimd.tensor_tensor(m13, odA, vsp, Alu.mult)
                nc.vector.tensor_scalar(m13, m13, 1.0, None, Alu.add)
                for h in (8, 4, 2):
                    nc.vector.tensor_tensor(
                        m1[:, 0:h * F], m1[:, 0:h * F], m1[:, h * F:2 * h * F],
                        Alu.mult)
                tD = sm.tile([P, F], DT, tag="tD")
                nc.vector.tensor_tensor(tD[:], m1[:, 0:F], m1[:, F:2 * F],
                                        Alu.mult)
                nc.vector.tensor_tensor(RS[:, F:2 * F], SPt[:, col], tD[:],
                                        Alu.mult)

            # One Exp op serves tanh(R*1e4), tanh(SP*1e4) and exp(min(R,100)):
            #   cols 0:2F  : t = exp(min(-2e4*x, 87))   [tanh arg, x = R | SP]
            #   cols 2F:3F : e = exp(min(R, 100))
            # tanh(x*1e4) = 2/(1+t) - 1  (exact for saturated x; the arg
            # clamp at 87 keeps t finite so the formula yields -1 there)
            X3 = sm.tile([P, 3 * F], DT, tag="X3")
            E3 = sm.tile([P, 3 * F], DT, tag="E3")
            nc.vector.tensor_scalar(X3[:, 0:2 * F], RS[:], NTANH, 87.0,
                                    Alu.mult, Alu.min)
            nc.vector.tensor_scalar(X3[:, 2 * F:3 * F], RS[:, 0:F], LOG_LIM,
                                    None, Alu.min)
            nc.scalar.activation(E3[:], X3[:], Act.Exp)
            D2 = sm.tile([P, 2 * F], DT, tag="D2")
            TH = sm.tile([P, 2 * F], DT, tag="TH")
            nc.vector.tensor_scalar(D2[:], E3[:, 0:2 * F], 1.0, None, Alu.add)
            nc.vector.reciprocal(D2[:], D2[:])
            nc.vector.tensor_scalar(TH[:], D2[:], 2.0, -1.0, Alu.mult, Alu.add)
            lin = TH[:, 0:F]
            lsig = TH[:, F:2 * F]

            # vs_new = g*lin + c*lsig
            tE = sm.tile([P, F], DT, tag="tE")
            tFt = sm.tile([P, F], DT, tag="tF")
            vsnew = sm.tile([P, F], DT, tag="vsnew")
            nc.vector.tensor_tensor(tE[:], gs, lin, Alu.mult)
            nc.vector.tensor_tensor(tFt[:], cs, lsig, Alu.mult)
            nc.vector.tensor_tensor(vsnew[:], tE[:], tFt[:], Alu.add)
            if s < S - 1:
                nc.vector.tensor_copy(vsh[:, col], vsnew[:])

            # vm_new = g*|R| + c*e ; |R| = max(R, -R) (abs_max is reduce-only)
            tG = sm.tile([P, F], DT, tag="tG")
            tH = sm.tile([P, F], DT, tag="tH")
            aR = sm.tile([P, F], DT, tag="aR")
            vmnew = sm.tile([P, F], DT, tag="vmnew")
            nc.vector.tensor_scalar(tG[:], RS[:, 0:F], -1.0, None, Alu.mult)
            nc.vector.tensor_tensor(aR[:], RS[:, 0:F], tG[:], Alu.max)
            nc.vector.tensor_tensor(tG[:], gs, aR[:], Alu.mult)
            nc.vector.tensor_tensor(tH[:], cs, E3[:, 2 * F:3 * F], Alu.mult)
            nc.vector.tensor_tensor(vmnew[:], tG[:], tH[:], Alu.add)

            if s == S - 1:
                nc.vector.tensor_tensor(outt[:], vsnew[:], vmnew[:], Alu.mult)
            else:
                nc.vector.tensor_tensor(sgh[:, col], vsnew[:], vmnew[:],
                                        Alu.mult)
                # lm = ln(clip(vm, 1e-12, FLT_MAX)), but ACT Ln only covers
                # |x| <= 2^64 while vm reaches FLT_MAX. Split off the float
                # exponent with integer bit ops: vm = m * 2^E, m in [1,2):
                #   lm = ln(m) + (E_biased - 127)*ln2
                # +inf vm is reconstructed to lm=+inf via mb; NaN vm positions
                # stay NaN downstream through the sg chain regardless of lm.
                tI = sm.tile([P, F], DT, tag="tI")
                tJ = sm.tile([P, F], DT, tag="tJ")
                tMm = sm.tile([P, F], DT, tag="tMm")
                tEx = sm.tile([P, F], DT, tag="tEx")
                mb = sm.tile([P, F], DT, tag="mb")
                nc.vector.tensor_scalar(tI[:], vmnew[:], 1e-12, FLT_MAX,
                                        Alu.max, Alu.min)
                tExU = sm.tile([P, F], mybir.dt.uint32, tag="tExU")
                tIu = tI[:].bitcast(mybir.dt.uint32)
                nc.vector.tensor_scalar(tExU[:], tIu, 23, None,
                                        Alu.logical_shift_right)
                nc.vector.tensor_scalar(
                    tMm[:].bitcast(mybir.dt.uint32), tIu,
                    0x007FFFFF, 0x3F800000, Alu.bitwise_and, Alu.bitwise_or)
                nc.scalar.activation(tJ[:], tMm[:], Act.Ln)
                # u32 -> f32 conversion fused into the affine (non-bitvec
                # ALU ops cast their input)
                nc.vector.tensor_scalar(tEx[:], tExU[:], LN2, -127.0 * LN2,
                                        Alu.mult, Alu.add)
                nc.vector.tensor_tensor(tJ[:], tJ[:], tEx[:], Alu.add)
                nc.vector.tensor_scalar(mb[:], vmnew[:], FLT_MAX, 0.0,
                                        Alu.subtract, Alu.max)
                nc.vector.tensor_tensor(lmh[:, col], tJ[:], mb[:], Alu.add)

        nc.sync.dma_start(out_d.ap(), outt[:])

    nc.compile()
    return nc


def pack_core(V_mag_b, V_sign_b, O_b, G_b, F):
    """Layout-only host prep for one core. All inputs [Tc,...] with Tc=128*F."""
    Tc = P * F
    vm0 = np.ascontiguousarray(V_mag_b[:, :16].reshape(P, F * 16))
    vs0 = np.ascontiguousarray(V_sign_b[:, :16].reshape(P, F * 16))
    g = np.ascontiguousarray(
        G_b.reshape(P, F, S).transpose(0, 2, 1).reshape(P, S * F))
    ost = np.ascontiguousarray(
        O_b[:, :, :16].reshape(P, F, S, 16).transpose(0, 2, 1, 3)
        .reshape(P, S * F * 16))
    cols = [O_b[:, s, 16:16 + s] for s in range(1, S)]
    tri = np.concatenate(cols, axis=1)  # [Tc, 120]
    ody = np.ascontiguousarray(
        tri.reshape(P, F, TRI).transpose(0, 2, 1).reshape(P, TRI * F))
    return {"vm0": vm0, "vs0": vs0, "g": g, "ost": ost, "ody": ody}


_PROGRAM = None


def _get_program():
    global _PROGRAM
    if _PROGRAM is None:
        _PROGRAM = build_program(T // P)
    return _PROGRAM


def kernel(V_mag, V_sign, O, G):
    V_mag = np.asarray(V_mag, f32)
    V_sign = np.asarray(V_sign, f32)
    O = np.asarray(O, f32)
    G = np.asarray(G, f32)
    F = T // P
    nc = _get_program()
    in_maps = [pack_core(V_mag[b], V_sign[b], O[b], G[b], F) for b in range(B)]
    res = run_bass_kernel_spmd(nc, in_maps, core_ids=list(range(B)))
    out = np.zeros((B, T), f32)
    for b in range(B):
        out[b] = res.results[b]["out"].reshape(T)
    return out


# revision 26
# speedup vs baseline: 1.0639x; 1.0639x over previous
"""Trainium2 Bass kernel for nn_DAGExecutor (B=8, T=4096, dag_depth=16, nodes=32).

Sharding: pure data parallel over the batch dim — batch b runs on core b.
Each core processes T=4096 positions laid out as [128 partitions x F=32].

Math restructure (numerically validated against the reference, incl. inf/NaN):
  For each position and step s, the node axis splits into
    - static nodes j<16 (initial V values, never overwritten, always finite):
      P[s] = sum_j O[s,j]*log(Vm0[j]),  Q[s] = sum_j O[s,j]*Vs0[j]*Vm0[j],
      SP0[s] = prod_j (Vs0[j]*2|O[s,j]|+1)   -- all precomputed in bulk ops
    - dynamic nodes j=16+s' (s'<s), written by the recurrence (may be inf):
      consumed per step with the reference's exact form
      mixed = (1-G)*lm + G*sg to reproduce inf/NaN propagation.
  R[s] = (1-G[s])*P[s] + G[s]*Q[s] + sum_{s'<s} Odyn[s,s']*mixed[s']
  sign_prod[s] = SP0[s] * prod_{s'<s} (vs[s']*2|Odyn[s,s']|+1)

tanh(x) is computed as sign(x)*(1-e)/(1+e), e=exp(-2|x|), so that every
activation (Exp/Ln/Sign) lives in the single ACT table set
natural_log_exp_and_others -- avoids per-step table reloads.

Host-side prep is layout-only: slicing, transposing, triangular packing.
The masked region of O (nodes >= 16+s at step s) is never shipped.
"""
import numpy as np
from contextlib import ExitStack

import concourse.bass as bass
import concourse.bacc as bacc
import concourse.mybir as mybir
from concourse.tile import TileContext
from concourse.bass_utils import run_bass_kernel_spmd

f32 = np.float32
DT = mybir.dt.float32
Alu = mybir.AluOpType
Act = mybir.ActivationFunctionType
AX = mybir.AxisListType

B, T, S, NODES = 8, 4096, 16, 32

_ACT_SET = "natural_log_exp_and_others"  # contains Exp, Ln, Sign


class _OneActSetBacc(bacc.Bacc):
    """Bacc whose act-table-load pass may only pick one function set.

    The stock pass greedily picks the first set containing each activation's
    function, which alternates exp_and_others / natural_log across the step
    loop (one ~2.7us table DMA per switch). Every function this kernel uses
    (Exp, Ln, Sign) lives in natural_log_exp_and_others, so restricting the
    candidate list to that set yields a single hoisted load. Indices into
    act_info.json's act_func_sets are preserved (walrus remaps by index).
    """

    def insert_act_table_loads(self):
        from concourse.hw_specs import get_activation_tables
        import bass_rust as _bass_rust
        has_activation = any(
            isinstance(i, mybir.InstActivation)
            for b in self.main_func.blocks for i in b.instructions)
        if not has_activation:
            return
        tables = [
            (name, (fns if name == _ACT_SET else set()))
            for name, fns in get_activation_tables(self.m.arch).items()
        ]
        _bass_rust.insert_act_table_loads(self, tables)
P = 128
TRI = S * (S - 1) // 2  # 120
OFF = [s * (s - 1) // 2 for s in range(S + 1)]
LOG_LIM = 100.0
NTANH = -2.0e4  # -2/TEMP
FLT_MAX = float(np.finfo(np.float32).max)
LN2 = float(np.log(np.float64(2.0)))


def build_program(F, n_devices=8):
    """Build + compile the per-core program for F positions per partition."""
    nc = _OneActSetBacc("TRN2", target_bir_lowering=False, debug=False,
                        num_devices=n_devices)
    vm0_d = nc.dram_tensor("vm0", [P, F * 16], DT, kind="ExternalInput")
    vs0_d = nc.dram_tensor("vs0", [P, F * 16], DT, kind="ExternalInput")
    g_d = nc.dram_tensor("g", [P, S * F], DT, kind="ExternalInput")
    ost_d = nc.dram_tensor("ost", [P, S * F * 16], DT, kind="ExternalInput")
    ody_d = nc.dram_tensor("ody", [P, TRI * F], DT, kind="ExternalInput")
    out_d = nc.dram_tensor("out", [P, F], DT, kind="ExternalOutput")

    SCHUNK = 8  # s-values per static chunk
    NCH = S // SCHUNK

    with TileContext(nc) as tc, ExitStack() as ctx:
        state = ctx.enter_context(tc.tile_pool(name="state", bufs=1))
        ost_pool = ctx.enter_context(tc.tile_pool(name="ost", bufs=2))
        big = ctx.enter_context(tc.tile_pool(name="big", bufs=2))
        sm = ctx.enter_context(tc.tile_pool(name="sm", bufs=3))

        # ---- persistent state tiles ----
        vm0 = state.tile([P, F * 16], DT, tag="vm0")
        vs0 = state.tile([P, F * 16], DT, tag="vs0")
        gt = state.tile([P, S * F], DT, tag="gt")
        ct = state.tile([P, S * F], DT, tag="ct")
        lm0 = state.tile([P, F * 16], DT, tag="lm0")
        sg0 = state.tile([P, F * 16], DT, tag="sg0")
        Pt = state.tile([P, S * F], DT, tag="Pt")
        Qt = state.tile([P, S * F], DT, tag="Qt")
        SPt = state.tile([P, S * F], DT, tag="SPt")
        lmh = state.tile([P, S * F], DT, tag="lmh")
        sgh = state.tile([P, S * F], DT, tag="sgh")
        vsh = state.tile([P, S * F], DT, tag="vsh")
        ody = state.tile([P, TRI * F], DT, tag="ody")
        odyA = state.tile([P, TRI * F], DT, tag="odyA")
        outt = state.tile([P, F], DT, tag="outt")

        nc.sync.dma_start(vm0[:], vm0_d.ap())
        nc.sync.dma_start(vs0[:], vs0_d.ap())
        nc.sync.dma_start(gt[:], g_d.ap())
        nc.sync.dma_start(ody[:], ody_d.ap())
        # 2*|O_dyn| for the sign-weight products, one bulk ACT op
        nc.scalar.activation(odyA[:], ody[:], Act.Abs, scale=2.0)

        # ---- stage A ----
        nc.vector.tensor_scalar(ct[:], gt[:], -1.0, 1.0, Alu.mult, Alu.add)
        nc.scalar.activation(lm0[:], vm0[:], Act.Ln)
        nc.vector.tensor_tensor(sg0[:], vs0[:], vm0[:], Alu.mult)

        lm0b = lm0[:].rearrange("p (f j) -> p f j", f=F, j=16) \
            .unsqueeze(1).to_broadcast([P, SCHUNK, F, 16])
        sg0b = sg0[:].rearrange("p (f j) -> p f j", f=F, j=16) \
            .unsqueeze(1).to_broadcast([P, SCHUNK, F, 16])
        vs0b = vs0[:].rearrange("p (f j) -> p f j", f=F, j=16) \
            .unsqueeze(1).to_broadcast([P, SCHUNK, F, 16])

        # ---- stage B: static prefix, per s-chunk ----
        for k in range(NCH):
            ostk = ost_pool.tile([P, SCHUNK * F * 16], DT, tag="ostk")
            nc.sync.dma_start(
                ostk[:], ost_d.ap()[:, k * SCHUNK * F * 16:(k + 1) * SCHUNK * F * 16])
            o4 = ostk[:].rearrange("p (s f j) -> p s f j", s=SCHUNK, f=F, j=16)
            cols = slice(k * SCHUNK * F, (k + 1) * SCHUNK * F)

            t1 = big.tile([P, SCHUNK * F * 16], DT, tag="t1")
            t14 = t1[:].rearrange("p (s f j) -> p s f j", s=SCHUNK, f=F, j=16)
            nc.vector.tensor_tensor(t14, o4, lm0b, Alu.mult)
            nc.vector.tensor_reduce(
                Pt[:, cols].rearrange("p (s f) -> p s f", s=SCHUNK, f=F),
                t14, AX.X, Alu.add)

            t2 = big.tile([P, SCHUNK * F * 16], DT, tag="t1")
            t24 = t2[:].rearrange("p (s f j) -> p s f j", s=SCHUNK, f=F, j=16)
            # (GPSIMD offload measured slower: POOL shares DVE's SBUF port)
            nc.vector.tensor_tensor(t24, o4, sg0b, Alu.mult)
            nc.vector.tensor_reduce(
                Qt[:, cols].rearrange("p (s f) -> p s f", s=SCHUNK, f=F),
                t24, AX.X, Alu.add)

            t3 = big.tile([P, SCHUNK * F * 16], DT, tag="t1")
            t34 = t3[:].rearrange("p (s f j) -> p s f j", s=SCHUNK, f=F, j=16)
            # w = vs0 * 2|O| + 1  (abs on ACT: abs_max is reduce-only ISA;
            # +1 on ACT keeps DVE free for the reduces)
            nc.scalar.activation(t34, o4, Act.Abs, scale=2.0)
            nc.vector.tensor_tensor(t34, t34, vs0b, Alu.mult)
            nc.scalar.activation(t34, t34, Act.Identity, bias=1.0)
            # product over j via a binary multiply tree (reduce-mult is not
            # supported by the DVE)
            for h in (8, 4, 2):
                nc.vector.tensor_tensor(
                    t34[:, :, :, 0:h], t34[:, :, :, 0:h], t34[:, :, :, h:2 * h],
                    Alu.mult)
            nc.vector.tensor_tensor(
                SPt[:, cols].rearrange("p (s f) -> p s f", s=SCHUNK, f=F)
                .unsqueeze(3),
                t34[:, :, :, 0:1], t34[:, :, :, 1:2], Alu.mult)

        # R static part for all steps at once: Rst = c*P + g*Q
        Rst = state.tile([P, S * F], DT, tag="Rst")
        nc.vector.tensor_tensor(Rst[:], ct[:], Pt[:], Alu.mult)
        nc.vector.tensor_tensor(Qt[:], gt[:], Qt[:], Alu.mult)
        nc.vector.tensor_tensor(Rst[:], Rst[:], Qt[:], Alu.add)

        # W-path work tile: rows s..15 hold 1.0 permanently (each step s
        # only writes rows 0..s-1, so a single upfront fill suffices);
        # the multiply tree writes into m2 to keep m1's padding intact.
        m1 = state.tile([P, 16 * F], DT, tag="m1")
        m2 = state.tile([P, 16 * F], DT, tag="m2")
        nc.gpsimd.memset(m1[:], 1.0)

        # ---- stage C: the 16-step recurrence ----
        for s in range(S):
            col = slice(s * F, (s + 1) * F)
            gs = gt[:, col]
            cs = ct[:, col]

            RS = sm.tile([P, 2 * F], DT, tag="RS")

            if s == 0:
                nc.vector.tensor_copy(RS[:, 0:F], Rst[:, col])
                nc.vector.tensor_copy(RS[:, F:2 * F], SPt[:, col])
            else:
                lmp = lmh[:, 0:s * F].rearrange("p (z f) -> p z f", z=s, f=F)
                sgp = sgh[:, 0:s * F].rearrange("p (z f) -> p z f", z=s, f=F)
                vsp = vsh[:, 0:s * F].rearrange("p (z f) -> p z f", z=s, f=F)
                cb = cs.unsqueeze(1).to_broadcast([P, s, F])
                gb = gs.unsqueeze(1).to_broadcast([P, s, F])
                od = ody[:, OFF[s] * F:(OFF[s] + s) * F] \
                    .rearrange("p (z f) -> p z f", z=s, f=F)

                m13 = m1[:, 0:s * F].rearrange("p (z f) -> p z f", z=s, f=F)
                m23 = m2[:, 0:s * F].rearrange("p (z f) -> p z f", z=s, f=F)
                # mixed = c*lm + g*sg  (reference's exact elementwise form)
                nc.vector.tensor_tensor(m13, lmp, cb, Alu.mult)
                nc.vector.tensor_tensor(m23, sgp, gb, Alu.mult)
                nc.vector.tensor_tensor(m13, m13, m23, Alu.add)
                nc.vector.tensor_tensor(m23, od, m13, Alu.mult)
                # reduce over s' (stride-F axis innermost)
                m23T = m2[:, 0:s * F].rearrange("p (z f) -> p f z", z=s, f=F)
                tC = sm.tile([P, F], DT, tag="tC")
                nc.vector.tensor_reduce(tC[:], m23T, AX.X, Alu.add)
                nc.vector.tensor_tensor(RS[:, 0:F], Rst[:, col], tC[:],
                                        Alu.add)
                # sign-weight dynamic product: rows s..15 are 1.0 (padding
                # kept intact across steps), binary multiply tree into m2
                odA = odyA[:, OFF[s] * F:(OFF[s] + s) * F] \
                    .rearrange("p (z f) -> p z f", z=s, f=F)
                nc.vector.tensor_tensor(m13, odA, vsp, Alu.mult)
                nc.vector.tensor_scalar(m13, m13, 1.0, None, Alu.add)
                nc.vector.tensor_tensor(
                    m2[:, 0:8 * F], m1[:, 0:8 * F], m1[:, 8 * F:16 * F],
                    Alu.mult)
                for h in (4, 2):
                    nc.vector.tensor_tensor(
                        m2[:, 0:h * F], m2[:, 0:h * F], m2[:, h * F:2 * h * F],
                        Alu.mult)
                tD = sm.tile([P, F], DT, tag="tD")
                nc.vector.tensor_tensor(tD[:], m2[:, 0:F], m2[:, F:2 * F],
                                        Alu.mult)
                nc.vector.tensor_tensor(RS[:, F:2 * F], SPt[:, col], tD[:],
                                        Alu.mult)

            # One Exp op serves tanh(R*1e4), tanh(SP*1e4) and exp(min(R,100)):
            #   cols 0:2F  : t = exp(min(-2e4*x, 87))   [tanh arg, x = R | SP]
            #   cols 2F:3F : e = exp(min(R, 100))
            # tanh(x*1e4) = 2/(1+t) - 1  (exact for saturated x; the arg
            # clamp at 87 keeps t finite so the formula yields -1 there)
            X3 = sm.tile([P, 3 * F], DT, tag="X3")
            E3 = sm.tile([P, 3 * F], DT, tag="E3")
            nc.vector.tensor_scalar(X3[:, 0:2 * F], RS[:], NTANH, 87.0,
                                    Alu.mult, Alu.min)
            nc.vector.tensor_scalar(X3[:, 2 * F:3 * F], RS[:, 0:F], LOG_LIM,
                                    None, Alu.min)
            nc.scalar.activation(E3[:], X3[:], Act.Exp)
            D2 = sm.tile([P, 2 * F], DT, tag="D2")
            TH = sm.tile([P, 2 * F], DT, tag="TH")
            nc.vector.tensor_scalar(D2[:], E3[:, 0:2 * F], 1.0, None, Alu.add)
            nc.vector.reciprocal(D2[:], D2[:])
            nc.vector.tensor_scalar(TH[:], D2[:], 2.0, -1.0, Alu.mult, Alu.add)
            lin = TH[:, 0:F]
            lsig = TH[:, F:2 * F]

            # vs_new = g*lin + c*lsig  (written straight into the history)
            tE = sm.tile([P, F], DT, tag="tE")
            tFt = sm.tile([P, F], DT, tag="tF")
            vsnew = sm.tile([P, F], DT, tag="vsnew")
            vsdst = vsh[:, col] if s < S - 1 else vsnew[:]
            nc.vector.tensor_tensor(tE[:], gs, lin, Alu.mult)
            nc.vector.tensor_tensor(tFt[:], cs, lsig, Alu.mult)
            nc.vector.tensor_tensor(vsdst, tE[:], tFt[:], Alu.add)

            # vm_new = g*|R| + c*e ; |R| = max(R, -R) (abs_max is reduce-only)
            tG = sm.tile([P, F], DT, tag="tG")
            tH = sm.tile([P, F], DT, tag="tH")
            aR = sm.tile([P, F], DT, tag="aR")
            vmnew = sm.tile([P, F], DT, tag="vmnew")
            nc.vector.tensor_scalar(tG[:], RS[:, 0:F], -1.0, None, Alu.mult)
            nc.vector.tensor_tensor(aR[:], RS[:, 0:F], tG[:], Alu.max)
            nc.vector.tensor_tensor(tG[:], gs, aR[:], Alu.mult)
            nc.vector.tensor_tensor(tH[:], cs, E3[:, 2 * F:3 * F], Alu.mult)
            nc.vector.tensor_tensor(vmnew[:], tG[:], tH[:], Alu.add)

            if s == S - 1:
                nc.vector.tensor_tensor(outt[:], vsnew[:], vmnew[:], Alu.mult)
            else:
                nc.vector.tensor_tensor(sgh[:, col], vsdst, vmnew[:],
                                        Alu.mult)
                # lm = ln(clip(vm, 1e-12, FLT_MAX)), but ACT Ln only covers
                # |x| <= 2^64 while vm reaches FLT_MAX. Split off the float
                # exponent with integer bit ops: vm = m * 2^E, m in [1,2):
                #   lm = ln(m) + (E_biased - 127)*ln2
                # +inf vm is reconstructed to lm=+inf via mb; NaN vm positions
                # stay NaN downstream through the sg chain regardless of lm.
                tI = sm.tile([P, F], DT, tag="tI")
                tJ = sm.tile([P, F], DT, tag="tJ")
                tMm = sm.tile([P, F], DT, tag="tMm")
                tEx = sm.tile([P, F], DT, tag="tEx")
                mb = sm.tile([P, F], DT, tag="mb")
                nc.vector.tensor_scalar(tI[:], vmnew[:], 1e-12, FLT_MAX,
                                        Alu.max, Alu.min)
                tExU = sm.tile([P, F], mybir.dt.uint32, tag="tExU")
                tIu = tI[:].bitcast(mybir.dt.uint32)
                nc.vector.tensor_scalar(tExU[:], tIu, 23, None,
                                        Alu.logical_shift_right)
                nc.vector.tensor_scalar(
                    tMm[:].bitcast(mybir.dt.uint32), tIu,
                    0x007FFFFF, 0x3F800000, Alu.bitwise_and, Alu.bitwise_or)
                nc.scalar.activation(tJ[:], tMm[:], Act.Ln)
                # u32 -> f32 conversion fused into the affine (non-bitvec
                # ALU ops cast their input)
                nc.vector.tensor_scalar(tEx[:], tExU[:], LN2, -127.0 * LN2,
                                        Alu.mult, Alu.add)
                nc.vector.tensor_tensor(tJ[:], tJ[:], tEx[:], Alu.add)
                nc.vector.tensor_scalar(mb[:], vmnew[:], FLT_MAX, 0.0,
                                        Alu.subtract, Alu.max)
                nc.vector.tensor_tensor(lmh[:, col], tJ[:], mb[:], Alu.add)

        nc.sync.dma_start(out_d.ap(), outt[:])

    nc.compile()
    return nc


def pack_core(V_mag_b, V_sign_b, O_b, G_b, F):
    """Layout-only host prep for one core. All inputs [Tc,...] with Tc=128*F."""
    Tc = P * F
    vm0 = np.ascontiguousarray(V_mag_b[:, :16].reshape(P, F * 16))
    vs0 = np.ascontiguousarray(V_sign_b[:, :16].reshape(P, F * 16))
    g = np.ascontiguousarray(
        G_b.reshape(P, F, S).transpose(0, 2, 1).reshape(P, S * F))
    ost = np.ascontiguousarray(
        O_b[:, :, :16].reshape(P, F, S, 16).transpose(0, 2, 1, 3)
        .reshape(P, S * F * 16))
    cols = [O_b[:, s, 16:16 + s] for s in range(1, S)]
    tri = np.concatenate(cols, axis=1)  # [Tc, 120]
    ody = np.ascontiguousarray(
        tri.reshape(P, F, TRI).transpose(0, 2, 1).reshape(P, TRI * F))
    return {"vm0": vm0, "vs0": vs0, "g": g, "ost": ost, "ody": ody}


_PROGRAM = None


def _get_program():
    global _PROGRAM
    if _PROGRAM is None:
        _PROGRAM = build_program(T // P)
    return _PROGRAM


def kernel(V_mag, V_sign, O, G):
    V_mag = np.asarray(V_mag, f32)
    V_sign = np.asarray(V_sign, f32)
    O = np.asarray(O, f32)
    G = np.asarray(G, f32)
    F = T // P
    nc = _get_program()
    in_maps = [pack_core(V_mag[b], V_sign[b], O[b], G[b], F) for b in range(B)]
    res = run_bass_kernel_spmd(nc, in_maps, core_ids=list(range(B)))
    out = np.zeros((B, T), f32)
    for b in range(B):
        out[b] = res.results[b]["out"].reshape(T)
    return out


# revision 30
# speedup vs baseline: 1.1037x; 1.0374x over previous
"""Trainium2 Bass kernel for nn_DAGExecutor (B=8, T=4096, dag_depth=16, nodes=32).

Sharding: pure data parallel over the batch dim — batch b runs on core b.
Each core processes T=4096 positions laid out as [128 partitions x F=32].

Math restructure (numerically validated against the reference, incl. inf/NaN):
  For each position and step s, the node axis splits into
    - static nodes j<16 (initial V values, never overwritten, always finite):
      P[s] = sum_j O[s,j]*log(Vm0[j]),  Q[s] = sum_j O[s,j]*Vs0[j]*Vm0[j],
      SP0[s] = prod_j (Vs0[j]*2|O[s,j]|+1)   -- all precomputed in bulk ops
    - dynamic nodes j=16+s' (s'<s), written by the recurrence (may be inf):
      consumed per step with the reference's exact form
      mixed = (1-G)*lm + G*sg to reproduce inf/NaN propagation.
  R[s] = (1-G[s])*P[s] + G[s]*Q[s] + sum_{s'<s} Odyn[s,s']*mixed[s']
  sign_prod[s] = SP0[s] * prod_{s'<s} (vs[s']*2|Odyn[s,s']|+1)

tanh(x) is computed as sign(x)*(1-e)/(1+e), e=exp(-2|x|), so that every
activation (Exp/Ln/Sign) lives in the single ACT table set
natural_log_exp_and_others -- avoids per-step table reloads.

Host-side prep is layout-only: slicing, transposing, triangular packing.
The masked region of O (nodes >= 16+s at step s) is never shipped.
"""
import numpy as np
from contextlib import ExitStack

import concourse.bass as bass
import concourse.bacc as bacc
import concourse.mybir as mybir
from concourse.tile import TileContext
from concourse.bass_utils import run_bass_kernel_spmd

f32 = np.float32
DT = mybir.dt.float32
Alu = mybir.AluOpType
Act = mybir.ActivationFunctionType
AX = mybir.AxisListType

B, T, S, NODES = 8, 4096, 16, 32

_ACT_SET = "natural_log_exp_and_others"  # contains Exp, Ln, Sign


class _OneActSetBacc(bacc.Bacc):
    """Bacc whose act-table-load pass may only pick one function set.

    The stock pass greedily picks the first set containing each activation's
    function, which alternates exp_and_others / natural_log across the step
    loop (one ~2.7us table DMA per switch). Every function this kernel uses
    (Exp, Ln, Sign) lives in natural_log_exp_and_others, so restricting the
    candidate list to that set yields a single hoisted load. Indices into
    act_info.json's act_func_sets are preserved (walrus remaps by index).
    """

    def insert_act_table_loads(self):
        from concourse.hw_specs import get_activation_tables
        import bass_rust as _bass_rust
        has_activation = any(
            isinstance(i, mybir.InstActivation)
            for b in self.main_func.blocks for i in b.instructions)
        if not has_activation:
            return
        tables = [
            (name, (fns if name == _ACT_SET else set()))
            for name, fns in get_activation_tables(self.m.arch).items()
        ]
        _bass_rust.insert_act_table_loads(self, tables)
P = 128
TRI = S * (S - 1) // 2  # 120
OFF = [s * (s - 1) // 2 for s in range(S + 1)]
LOG_LIM = 100.0
NTANH = -2.0e4  # -2/TEMP
FLT_MAX = float(np.finfo(np.float32).max)
LN2 = float(np.log(np.float64(2.0)))


def build_program(F, n_devices=8):
    """Build + compile the per-core program for F positions per partition."""
    nc = _OneActSetBacc("TRN2", target_bir_lowering=False, debug=False,
                        num_devices=n_devices)
    vm0_d = nc.dram_tensor("vm0", [P, F * 16], DT, kind="ExternalInput")
    vs0_d = nc.dram_tensor("vs0", [P, F * 16], DT, kind="ExternalInput")
    g_d = nc.dram_tensor("g", [P, S * F], DT, kind="ExternalInput")
    ost_d = nc.dram_tensor("ost", [P, S * F * 16], DT, kind="ExternalInput")
    ody_d = nc.dram_tensor("ody", [P, TRI * F], DT, kind="ExternalInput")
    out_d = nc.dram_tensor("out", [P, F], DT, kind="ExternalOutput")

    SCHUNK = 4  # s-values per static chunk (small first chunk -> compute
    NCH = S // SCHUNK  # starts as soon as ~1MB of O has landed)

    with TileContext(nc) as tc, ExitStack() as ctx:
        state = ctx.enter_context(tc.tile_pool(name="state", bufs=1))
        ost_pool = ctx.enter_context(tc.tile_pool(name="ost", bufs=2))
        big = ctx.enter_context(tc.tile_pool(name="big", bufs=2))
        sm = ctx.enter_context(tc.tile_pool(name="sm", bufs=3))

        # ---- persistent state tiles ----
        vm0 = state.tile([P, F * 16], DT, tag="vm0")
        vs0 = state.tile([P, F * 16], DT, tag="vs0")
        gt = state.tile([P, S * F], DT, tag="gt")
        ct = state.tile([P, S * F], DT, tag="ct")
        lm0 = state.tile([P, F * 16], DT, tag="lm0")
        sg0 = state.tile([P, F * 16], DT, tag="sg0")
        Pt = state.tile([P, S * F], DT, tag="Pt")
        Qt = state.tile([P, S * F], DT, tag="Qt")
        SPt = state.tile([P, S * F], DT, tag="SPt")
        lmh = state.tile([P, S * F], DT, tag="lmh")
        sgh = state.tile([P, S * F], DT, tag="sgh")
        vsh = state.tile([P, S * F], DT, tag="vsh")
        ody = state.tile([P, TRI * F], DT, tag="ody")
        odyA = state.tile([P, TRI * F], DT, tag="odyA")
        outt = state.tile([P, F], DT, tag="outt")

        # DMA order matters: the first ost chunk gates the first big DVE op,
        # so it goes right after the small V tiles; ody is not needed until
        # step 1 of the recurrence (~50us later).
        nc.sync.dma_start(vm0[:], vm0_d.ap())
        nc.sync.dma_start(vs0[:], vs0_d.ap())

        # ---- stage A ----
        nc.scalar.activation(lm0[:], vm0[:], Act.Ln)
        nc.vector.tensor_tensor(sg0[:], vs0[:], vm0[:], Alu.mult)

        lm0b = lm0[:].rearrange("p (f j) -> p f j", f=F, j=16) \
            .unsqueeze(1).to_broadcast([P, SCHUNK, F, 16])
        sg0b = sg0[:].rearrange("p (f j) -> p f j", f=F, j=16) \
            .unsqueeze(1).to_broadcast([P, SCHUNK, F, 16])
        vs0b = vs0[:].rearrange("p (f j) -> p f j", f=F, j=16) \
            .unsqueeze(1).to_broadcast([P, SCHUNK, F, 16])

        # ---- stage B: static prefix, per s-chunk ----
        for k in range(NCH):
            ostk = ost_pool.tile([P, SCHUNK * F * 16], DT, tag="ostk")
            nc.sync.dma_start(
                ostk[:], ost_d.ap()[:, k * SCHUNK * F * 16:(k + 1) * SCHUNK * F * 16])
            o4 = ostk[:].rearrange("p (s f j) -> p s f j", s=SCHUNK, f=F, j=16)
            cols = slice(k * SCHUNK * F, (k + 1) * SCHUNK * F)

            t1 = big.tile([P, SCHUNK * F * 16], DT, tag="t1")
            t14 = t1[:].rearrange("p (s f j) -> p s f j", s=SCHUNK, f=F, j=16)
            nc.vector.tensor_tensor(t14, o4, lm0b, Alu.mult)
            nc.vector.tensor_reduce(
                Pt[:, cols].rearrange("p (s f) -> p s f", s=SCHUNK, f=F),
                t14, AX.X, Alu.add)

            t2 = big.tile([P, SCHUNK * F * 16], DT, tag="t1")
            t24 = t2[:].rearrange("p (s f j) -> p s f j", s=SCHUNK, f=F, j=16)
            # (GPSIMD offload measured slower: POOL shares DVE's SBUF port)
            nc.vector.tensor_tensor(t24, o4, sg0b, Alu.mult)
            nc.vector.tensor_reduce(
                Qt[:, cols].rearrange("p (s f) -> p s f", s=SCHUNK, f=F),
                t24, AX.X, Alu.add)

            t3 = big.tile([P, SCHUNK * F * 16], DT, tag="t1")
            t34 = t3[:].rearrange("p (s f j) -> p s f j", s=SCHUNK, f=F, j=16)
            # w = vs0 * 2|O| + 1  (abs on ACT: abs_max is reduce-only ISA;
            # +1 on ACT keeps DVE free for the reduces)
            nc.scalar.activation(t34, o4, Act.Abs, scale=2.0)
            nc.vector.tensor_tensor(t34, t34, vs0b, Alu.mult)
            nc.scalar.activation(t34, t34, Act.Identity, bias=1.0)
            # product over j via a binary multiply tree (reduce-mult is not
            # supported by the DVE)
            for h in (8, 4, 2):
                nc.vector.tensor_tensor(
                    t34[:, :, :, 0:h], t34[:, :, :, 0:h], t34[:, :, :, h:2 * h],
                    Alu.mult)
            nc.vector.tensor_tensor(
                SPt[:, cols].rearrange("p (s f) -> p s f", s=SCHUNK, f=F)
                .unsqueeze(3),
                t34[:, :, :, 0:1], t34[:, :, :, 1:2], Alu.mult)

        # late DMAs: not needed until the recurrence, keep them off the
        # critical startup path
        nc.sync.dma_start(gt[:], g_d.ap())
        nc.sync.dma_start(ody[:], ody_d.ap())
        nc.vector.tensor_scalar(ct[:], gt[:], -1.0, 1.0, Alu.mult, Alu.add)
        # 2*|O_dyn| for the sign-weight products, one bulk ACT op
        nc.scalar.activation(odyA[:], ody[:], Act.Abs, scale=2.0)

        # R static part for all steps at once: Rst = c*P + g*Q
        Rst = state.tile([P, S * F], DT, tag="Rst")
        nc.vector.tensor_tensor(Rst[:], ct[:], Pt[:], Alu.mult)
        nc.vector.tensor_tensor(Qt[:], gt[:], Qt[:], Alu.mult)
        nc.vector.tensor_tensor(Rst[:], Rst[:], Qt[:], Alu.add)

        # W-path work tile: rows s..15 hold 1.0 permanently (each step s
        # only writes rows 0..s-1, so a single upfront fill suffices);
        # the multiply tree writes into m2 to keep m1's padding intact.
        m1 = state.tile([P, 16 * F], DT, tag="m1")
        m2 = state.tile([P, 16 * F], DT, tag="m2")
        nc.gpsimd.memset(m1[:], 1.0)

        # ---- stage C: the 16-step recurrence ----
        for s in range(S):
            col = slice(s * F, (s + 1) * F)
            gs = gt[:, col]
            cs = ct[:, col]

            RS = sm.tile([P, 2 * F], DT, tag="RS")

            if s == 0:
                nc.vector.tensor_copy(RS[:, 0:F], Rst[:, col])
                nc.vector.tensor_copy(RS[:, F:2 * F], SPt[:, col])
            else:
                lmp = lmh[:, 0:s * F].rearrange("p (z f) -> p z f", z=s, f=F)
                sgp = sgh[:, 0:s * F].rearrange("p (z f) -> p z f", z=s, f=F)
                vsp = vsh[:, 0:s * F].rearrange("p (z f) -> p z f", z=s, f=F)
                cb = cs.unsqueeze(1).to_broadcast([P, s, F])
                gb = gs.unsqueeze(1).to_broadcast([P, s, F])
                od = ody[:, OFF[s] * F:(OFF[s] + s) * F] \
                    .rearrange("p (z f) -> p z f", z=s, f=F)

                m13 = m1[:, 0:s * F].rearrange("p (z f) -> p z f", z=s, f=F)
                m23 = m2[:, 0:s * F].rearrange("p (z f) -> p z f", z=s, f=F)
                # mixed = c*lm + g*sg  (reference's exact elementwise form)
                nc.vector.tensor_tensor(m13, lmp, cb, Alu.mult)
                nc.vector.tensor_tensor(m23, sgp, gb, Alu.mult)
                nc.vector.tensor_tensor(m13, m13, m23, Alu.add)
                nc.vector.tensor_tensor(m23, od, m13, Alu.mult)
                # reduce over s' (stride-F axis innermost)
                m23T = m2[:, 0:s * F].rearrange("p (z f) -> p f z", z=s, f=F)
                tC = sm.tile([P, F], DT, tag="tC")
                nc.vector.tensor_reduce(tC[:], m23T, AX.X, Alu.add)
                nc.vector.tensor_tensor(RS[:, 0:F], Rst[:, col], tC[:],
                                        Alu.add)
                # sign-weight dynamic product: rows s..15 are 1.0 (padding
                # kept intact across steps), binary multiply tree into m2
                odA = odyA[:, OFF[s] * F:(OFF[s] + s) * F] \
                    .rearrange("p (z f) -> p z f", z=s, f=F)
                nc.vector.tensor_tensor(m13, odA, vsp, Alu.mult)
                nc.vector.tensor_scalar(m13, m13, 1.0, None, Alu.add)
                nc.vector.tensor_tensor(
                    m2[:, 0:8 * F], m1[:, 0:8 * F], m1[:, 8 * F:16 * F],
                    Alu.mult)
                for h in (4, 2):
                    nc.vector.tensor_tensor(
                        m2[:, 0:h * F], m2[:, 0:h * F], m2[:, h * F:2 * h * F],
                        Alu.mult)
                tD = sm.tile([P, F], DT, tag="tD")
                nc.vector.tensor_tensor(tD[:], m2[:, 0:F], m2[:, F:2 * F],
                                        Alu.mult)
                nc.vector.tensor_tensor(RS[:, F:2 * F], SPt[:, col], tD[:],
                                        Alu.mult)

            # One Exp op serves tanh(R*1e4), tanh(SP*1e4) and exp(min(R,100)):
            #   cols 0:2F  : t = exp(min(-2e4*x, 87))   [tanh arg, x = R | SP]
            #   cols 2F:3F : e = exp(min(R, 100))
            # tanh(x*1e4) = 2/(1+t) - 1  (exact for saturated x; the arg
            # clamp at 87 keeps t finite so the formula yields -1 there)
            X3 = sm.tile([P, 3 * F], DT, tag="X3")
            E3 = sm.tile([P, 3 * F], DT, tag="E3")
            nc.vector.tensor_scalar(X3[:, 0:2 * F], RS[:], NTANH, 87.0,
                                    Alu.mult, Alu.min)
            nc.vector.tensor_scalar(X3[:, 2 * F:3 * F], RS[:, 0:F], LOG_LIM,
                                    None, Alu.min)
            nc.scalar.activation(E3[:], X3[:], Act.Exp)
            D2 = sm.tile([P, 2 * F], DT, tag="D2")
            TH = sm.tile([P, 2 * F], DT, tag="TH")
            nc.vector.tensor_scalar(D2[:], E3[:, 0:2 * F], 1.0, None, Alu.add)
            nc.vector.reciprocal(D2[:], D2[:])
            nc.vector.tensor_scalar(TH[:], D2[:], 2.0, -1.0, Alu.mult, Alu.add)
            lin = TH[:, 0:F]
            lsig = TH[:, F:2 * F]

            # vs_new = g*lin + c*lsig  (written straight into the history)
            tE = sm.tile([P, F], DT, tag="tE")
            tFt = sm.tile([P, F], DT, tag="tF")
            vsnew = sm.tile([P, F], DT, tag="vsnew")
            vsdst = vsh[:, col] if s < S - 1 else vsnew[:]
            nc.vector.tensor_tensor(tE[:], gs, lin, Alu.mult)
            nc.vector.tensor_tensor(tFt[:], cs, lsig, Alu.mult)
            nc.vector.tensor_tensor(vsdst, tE[:], tFt[:], Alu.add)

            # vm_new = g*|R| + c*e ; |R| = max(R, -R) (abs_max is reduce-only)
            tG = sm.tile([P, F], DT, tag="tG")
            tH = sm.tile([P, F], DT, tag="tH")
            aR = sm.tile([P, F], DT, tag="aR")
            vmnew = sm.tile([P, F], DT, tag="vmnew")
            nc.vector.tensor_scalar(tG[:], RS[:, 0:F], -1.0, None, Alu.mult)
            nc.vector.tensor_tensor(aR[:], RS[:, 0:F], tG[:], Alu.max)
            nc.vector.tensor_tensor(tG[:], gs, aR[:], Alu.mult)
            nc.vector.tensor_tensor(tH[:], cs, E3[:, 2 * F:3 * F], Alu.mult)
            nc.vector.tensor_tensor(vmnew[:], tG[:], tH[:], Alu.add)

            if s == S - 1:
                nc.vector.tensor_tensor(outt[:], vsnew[:], vmnew[:], Alu.mult)
            else:
                nc.vector.tensor_tensor(sgh[:, col], vsdst, vmnew[:],
                                        Alu.mult)
                # lm = ln(clip(vm, 1e-12, FLT_MAX)), but ACT Ln only covers
                # |x| <= 2^64 while vm reaches FLT_MAX. Split off the float
                # exponent with integer bit ops: vm = m * 2^E, m in [1,2):
                #   lm = ln(m) + (E_biased - 127)*ln2
                # +inf vm is reconstructed to lm=+inf via mb; NaN vm positions
                # stay NaN downstream through the sg chain regardless of lm.
                tI = sm.tile([P, F], DT, tag="tI")
                tJ = sm.tile([P, F], DT, tag="tJ")
                tMm = sm.tile([P, F], DT, tag="tMm")
                tEx = sm.tile([P, F], DT, tag="tEx")
                mb = sm.tile([P, F], DT, tag="mb")
                nc.vector.tensor_scalar(tI[:], vmnew[:], 1e-12, FLT_MAX,
                                        Alu.max, Alu.min)
                tExU = sm.tile([P, F], mybir.dt.uint32, tag="tExU")
                tIu = tI[:].bitcast(mybir.dt.uint32)
                nc.vector.tensor_scalar(tExU[:], tIu, 23, None,
                                        Alu.logical_shift_right)
                nc.vector.tensor_scalar(
                    tMm[:].bitcast(mybir.dt.uint32), tIu,
                    0x007FFFFF, 0x3F800000, Alu.bitwise_and, Alu.bitwise_or)
                nc.scalar.activation(tJ[:], tMm[:], Act.Ln)
                # u32 -> f32 conversion fused into the affine (non-bitvec
                # ALU ops cast their input)
                nc.vector.tensor_scalar(tEx[:], tExU[:], LN2, -127.0 * LN2,
                                        Alu.mult, Alu.add)
                nc.vector.tensor_tensor(tJ[:], tJ[:], tEx[:], Alu.add)
                nc.vector.tensor_scalar(mb[:], vmnew[:], FLT_MAX, 0.0,
                                        Alu.subtract, Alu.max)
                nc.vector.tensor_tensor(lmh[:, col], tJ[:], mb[:], Alu.add)

        nc.sync.dma_start(out_d.ap(), outt[:])

    nc.compile()
    return nc


def pack_core(V_mag_b, V_sign_b, O_b, G_b, F):
    """Layout-only host prep for one core. All inputs [Tc,...] with Tc=128*F."""
    Tc = P * F
    vm0 = np.ascontiguousarray(V_mag_b[:, :16].reshape(P, F * 16))
    vs0 = np.ascontiguousarray(V_sign_b[:, :16].reshape(P, F * 16))
    g = np.ascontiguousarray(
        G_b.reshape(P, F, S).transpose(0, 2, 1).reshape(P, S * F))
    ost = np.ascontiguousarray(
        O_b[:, :, :16].reshape(P, F, S, 16).transpose(0, 2, 1, 3)
        .reshape(P, S * F * 16))
    cols = [O_b[:, s, 16:16 + s] for s in range(1, S)]
    tri = np.concatenate(cols, axis=1)  # [Tc, 120]
    ody = np.ascontiguousarray(
        tri.reshape(P, F, TRI).transpose(0, 2, 1).reshape(P, TRI * F))
    return {"vm0": vm0, "vs0": vs0, "g": g, "ost": ost, "ody": ody}


_PROGRAM = None


def _get_program():
    global _PROGRAM
    if _PROGRAM is None:
        _PROGRAM = build_program(T // P)
    return _PROGRAM


def kernel(V_mag, V_sign, O, G):
    V_mag = np.asarray(V_mag, f32)
    V_sign = np.asarray(V_sign, f32)
    O = np.asarray(O, f32)
    G = np.asarray(G, f32)
    F = T // P
    nc = _get_program()
    in_maps = [pack_core(V_mag[b], V_sign[b], O[b], G[b], F) for b in range(B)]
    res = run_bass_kernel_spmd(nc, in_maps, core_ids=list(range(B)))
    out = np.zeros((B, T), f32)
    for b in range(B):
        out[b] = res.results[b]["out"].reshape(T)
    return out


# revision 35
# speedup vs baseline: 1.1175x; 1.0125x over previous
"""Trainium2 Bass kernel for nn_DAGExecutor (B=8, T=4096, dag_depth=16, nodes=32).

Sharding: pure data parallel over the batch dim — batch b runs on core b.
Each core processes T=4096 positions laid out as [128 partitions x F=32].

Math restructure (numerically validated against the reference, incl. inf/NaN):
  For each position and step s, the node axis splits into
    - static nodes j<16 (initial V values, never overwritten, always finite):
      P[s] = sum_j O[s,j]*log(Vm0[j]),  Q[s] = sum_j O[s,j]*Vs0[j]*Vm0[j],
      SP0[s] = prod_j (Vs0[j]*2|O[s,j]|+1)   -- all precomputed in bulk ops
    - dynamic nodes j=16+s' (s'<s), written by the recurrence (may be inf):
      consumed per step with the reference's exact form
      mixed = (1-G)*lm + G*sg to reproduce inf/NaN propagation.
  R[s] = (1-G[s])*P[s] + G[s]*Q[s] + sum_{s'<s} Odyn[s,s']*mixed[s']
  sign_prod[s] = SP0[s] * prod_{s'<s} (vs[s']*2|Odyn[s,s']|+1)

tanh(x) is computed as sign(x)*(1-e)/(1+e), e=exp(-2|x|), so that every
activation (Exp/Ln/Sign) lives in the single ACT table set
natural_log_exp_and_others -- avoids per-step table reloads.

Host-side prep is layout-only: slicing, transposing, triangular packing.
The masked region of O (nodes >= 16+s at step s) is never shipped.
"""
import numpy as np
from contextlib import ExitStack

import concourse.bass as bass
import concourse.bacc as bacc
import concourse.mybir as mybir
from concourse.tile import TileContext
from concourse.bass_utils import run_bass_kernel_spmd

f32 = np.float32
DT = mybir.dt.float32
Alu = mybir.AluOpType
Act = mybir.ActivationFunctionType
AX = mybir.AxisListType

B, T, S, NODES = 8, 4096, 16, 32

_ACT_SET = "natural_log_exp_and_others"  # contains Exp, Ln, Sign


class _OneActSetBacc(bacc.Bacc):
    """Bacc whose act-table-load pass may only pick one function set.

    The stock pass greedily picks the first set containing each activation's
    function, which alternates exp_and_others / natural_log across the step
    loop (one ~2.7us table DMA per switch). Every function this kernel uses
    (Exp, Ln, Sign) lives in natural_log_exp_and_others, so restricting the
    candidate list to that set yields a single hoisted load. Indices into
    act_info.json's act_func_sets are preserved (walrus remaps by index).
    """

    def insert_act_table_loads(self):
        from concourse.hw_specs import get_activation_tables
        import bass_rust as _bass_rust
        has_activation = any(
            isinstance(i, mybir.InstActivation)
            for b in self.main_func.blocks for i in b.instructions)
        if not has_activation:
            return
        tables = [
            (name, (fns if name == _ACT_SET else set()))
            for name, fns in get_activation_tables(self.m.arch).items()
        ]
        _bass_rust.insert_act_table_loads(self, tables)
P = 128
TRI = S * (S - 1) // 2  # 120
OFF = [s * (s - 1) // 2 for s in range(S + 1)]
LOG_LIM = 100.0
NTANH = -2.0e4  # -2/TEMP
FLT_MAX = float(np.finfo(np.float32).max)
LN2 = float(np.log(np.float64(2.0)))


def build_program(F, n_devices=8):
    """Build + compile the per-core program for F positions per partition."""
    nc = _OneActSetBacc("TRN2", target_bir_lowering=False, debug=False,
                        num_devices=n_devices)
    vm0_d = nc.dram_tensor("vm0", [P, F * 16], DT, kind="ExternalInput")
    vs0_d = nc.dram_tensor("vs0", [P, F * 16], DT, kind="ExternalInput")
    g_d = nc.dram_tensor("g", [P, S * F], DT, kind="ExternalInput")
    ost_d = nc.dram_tensor("ost", [P, S * F * 16], DT, kind="ExternalInput")
    ody_d = nc.dram_tensor("ody", [P, TRI * F], DT, kind="ExternalInput")
    out_d = nc.dram_tensor("out", [P, F], DT, kind="ExternalOutput")

    SCHUNK = 4  # s-values per static chunk (small first chunk -> compute
    NCH = S // SCHUNK  # starts as soon as ~1MB of O has landed)

    with TileContext(nc) as tc, ExitStack() as ctx:
        state = ctx.enter_context(tc.tile_pool(name="state", bufs=1))
        # all NCH ost chunks stay live until the second (W-path) pass reads
        # them, so the pool needs one buffer per chunk
        ost_pool = ctx.enter_context(tc.tile_pool(name="ost", bufs=4))
        big = ctx.enter_context(tc.tile_pool(name="big", bufs=2))
        sm = ctx.enter_context(tc.tile_pool(name="sm", bufs=4))

        # ---- persistent state tiles ----
        vm0 = state.tile([P, F * 16], DT, tag="vm0")
        vs0 = state.tile([P, F * 16], DT, tag="vs0")
        gt = state.tile([P, S * F], DT, tag="gt")
        ct = state.tile([P, S * F], DT, tag="ct")
        lm0 = state.tile([P, F * 16], DT, tag="lm0")
        sg0 = state.tile([P, F * 16], DT, tag="sg0")
        Pt = state.tile([P, S * F], DT, tag="Pt")
        Qt = state.tile([P, S * F], DT, tag="Qt")
        SPt = state.tile([P, S * F], DT, tag="SPt")
        lmh = state.tile([P, S * F], DT, tag="lmh")
        sgh = state.tile([P, S * F], DT, tag="sgh")
        vsh = state.tile([P, S * F], DT, tag="vsh")
        ody = state.tile([P, TRI * F], DT, tag="ody")
        odyA = state.tile([P, TRI * F], DT, tag="odyA")
        outt = state.tile([P, F], DT, tag="outt")

        # DMA order matters: the first ost chunk gates the first big DVE op,
        # so it goes right after the small V tiles; ody is not needed until
        # step 1 of the recurrence (~50us later).
        nc.sync.dma_start(vm0[:], vm0_d.ap())
        nc.sync.dma_start(vs0[:], vs0_d.ap())

        # ---- stage A ----
        nc.scalar.activation(lm0[:], vm0[:], Act.Ln)
        nc.vector.tensor_tensor(sg0[:], vs0[:], vm0[:], Alu.mult)

        lm0b = lm0[:].rearrange("p (f j) -> p f j", f=F, j=16) \
            .unsqueeze(1).to_broadcast([P, SCHUNK, F, 16])
        sg0b = sg0[:].rearrange("p (f j) -> p f j", f=F, j=16) \
            .unsqueeze(1).to_broadcast([P, SCHUNK, F, 16])
        vs0b = vs0[:].rearrange("p (f j) -> p f j", f=F, j=16) \
            .unsqueeze(1).to_broadcast([P, SCHUNK, F, 16])

        # ---- stage B: static prefix, per s-chunk ----
        # Two emission passes: DVE executes its stream in order, so the
        # W-path (gated on ACT Abs) must not sit between ready P/Q work.
        ost_tiles = []
        for k in range(NCH):
            ostk = ost_pool.tile([P, SCHUNK * F * 16], DT, tag="ostk",
                                 name="ostk%d" % k)
            nc.sync.dma_start(
                ostk[:], ost_d.ap()[:, k * SCHUNK * F * 16:(k + 1) * SCHUNK * F * 16])
            ost_tiles.append(ostk)
            o4 = ostk[:].rearrange("p (s f j) -> p s f j", s=SCHUNK, f=F, j=16)
            cols = slice(k * SCHUNK * F, (k + 1) * SCHUNK * F)

            t1 = big.tile([P, SCHUNK * F * 16], DT, tag="t1")
            t14 = t1[:].rearrange("p (s f j) -> p s f j", s=SCHUNK, f=F, j=16)
            nc.vector.tensor_tensor(t14, o4, lm0b, Alu.mult)
            nc.vector.tensor_reduce(
                Pt[:, cols].rearrange("p (s f) -> p s f", s=SCHUNK, f=F),
                t14, AX.X, Alu.add)

            t2 = big.tile([P, SCHUNK * F * 16], DT, tag="t1")
            t24 = t2[:].rearrange("p (s f j) -> p s f j", s=SCHUNK, f=F, j=16)
            # (GPSIMD offload measured slower: POOL shares DVE's SBUF port)
            nc.vector.tensor_tensor(t24, o4, sg0b, Alu.mult)
            nc.vector.tensor_reduce(
                Qt[:, cols].rearrange("p (s f) -> p s f", s=SCHUNK, f=F),
                t24, AX.X, Alu.add)

        for k in range(NCH):
            o4 = ost_tiles[k][:].rearrange("p (s f j) -> p s f j", s=SCHUNK,
                                           f=F, j=16)
            cols = slice(k * SCHUNK * F, (k + 1) * SCHUNK * F)
            t3 = big.tile([P, SCHUNK * F * 16], DT, tag="t3")
            t34 = t3[:].rearrange("p (s f j) -> p s f j", s=SCHUNK, f=F, j=16)
            # w = vs0 * 2|O| + 1  (abs on ACT: abs_max is reduce-only ISA;
            # +1 on ACT keeps DVE free for the reduces)
            nc.scalar.activation(t34, o4, Act.Abs, scale=2.0)
            nc.vector.tensor_tensor(t34, t34, vs0b, Alu.mult)
            nc.scalar.activation(t34, t34, Act.Identity, bias=1.0)
            # product over j via a binary multiply tree (reduce-mult is not
            # supported by the DVE)
            for h in (8, 4, 2):
                nc.vector.tensor_tensor(
                    t34[:, :, :, 0:h], t34[:, :, :, 0:h], t34[:, :, :, h:2 * h],
                    Alu.mult)
            nc.vector.tensor_tensor(
                SPt[:, cols].rearrange("p (s f) -> p s f", s=SCHUNK, f=F)
                .unsqueeze(3),
                t34[:, :, :, 0:1], t34[:, :, :, 1:2], Alu.mult)

        # late DMAs: not needed until the recurrence, keep them off the
        # critical startup path
        nc.sync.dma_start(gt[:], g_d.ap())
        nc.sync.dma_start(ody[:], ody_d.ap())
        nc.vector.tensor_scalar(ct[:], gt[:], -1.0, 1.0, Alu.mult, Alu.add)
        # 2*|O_dyn| for the sign-weight products, one bulk ACT op
        nc.scalar.activation(odyA[:], ody[:], Act.Abs, scale=2.0)

        # R static part for all steps at once: Rst = c*P + g*Q
        Rst = state.tile([P, S * F], DT, tag="Rst")
        nc.vector.tensor_tensor(Rst[:], ct[:], Pt[:], Alu.mult)
        nc.vector.tensor_tensor(Qt[:], gt[:], Qt[:], Alu.mult)
        nc.vector.tensor_tensor(Rst[:], Rst[:], Qt[:], Alu.add)

        # W-path work tile: rows s..15 hold 1.0 permanently (each step s
        # only writes rows 0..s-1, so a single upfront fill suffices);
        # the multiply tree writes into m2 to keep m1's padding intact.
        m1 = state.tile([P, 16 * F], DT, tag="m1")
        m2 = state.tile([P, 16 * F], DT, tag="m2")
        nc.gpsimd.memset(m1[:], 1.0)

        # ---- stage C: the 16-step recurrence ----
        for s in range(S):
            col = slice(s * F, (s + 1) * F)
            gs = gt[:, col]
            cs = ct[:, col]

            RS = sm.tile([P, 2 * F], DT, tag="RS")

            if s == 0:
                pass  # step 0 reads Rst/SPt slices directly (no dyn part)
            else:
                lmp = lmh[:, 0:s * F].rearrange("p (z f) -> p z f", z=s, f=F)
                sgp = sgh[:, 0:s * F].rearrange("p (z f) -> p z f", z=s, f=F)
                vsp = vsh[:, 0:s * F].rearrange("p (z f) -> p z f", z=s, f=F)
                cb = cs.unsqueeze(1).to_broadcast([P, s, F])
                gb = gs.unsqueeze(1).to_broadcast([P, s, F])
                od = ody[:, OFF[s] * F:(OFF[s] + s) * F] \
                    .rearrange("p (z f) -> p z f", z=s, f=F)

                m13 = m1[:, 0:s * F].rearrange("p (z f) -> p z f", z=s, f=F)
                m23 = m2[:, 0:s * F].rearrange("p (z f) -> p z f", z=s, f=F)
                # mixed = c*lm + g*sg  (reference's exact elementwise form)
                nc.vector.tensor_tensor(m13, lmp, cb, Alu.mult)
                nc.vector.tensor_tensor(m23, sgp, gb, Alu.mult)
                nc.vector.tensor_tensor(m13, m13, m23, Alu.add)
                nc.vector.tensor_tensor(m23, od, m13, Alu.mult)
                # reduce over s' (stride-F axis innermost)
                m23T = m2[:, 0:s * F].rearrange("p (z f) -> p f z", z=s, f=F)
                tC = sm.tile([P, F], DT, tag="tC")
                nc.vector.tensor_reduce(tC[:], m23T, AX.X, Alu.add)
                nc.vector.tensor_tensor(RS[:, 0:F], Rst[:, col], tC[:],
                                        Alu.add)
                # sign-weight dynamic product: rows s..15 are 1.0 (padding
                # kept intact across steps), binary multiply tree into m2
                odA = odyA[:, OFF[s] * F:(OFF[s] + s) * F] \
                    .rearrange("p (z f) -> p z f", z=s, f=F)
                nc.vector.tensor_tensor(m13, odA, vsp, Alu.mult)
                nc.vector.tensor_scalar(m13, m13, 1.0, None, Alu.add)
                nc.vector.tensor_tensor(
                    m2[:, 0:8 * F], m1[:, 0:8 * F], m1[:, 8 * F:16 * F],
                    Alu.mult)
                for h in (4, 2):
                    nc.vector.tensor_tensor(
                        m2[:, 0:h * F], m2[:, 0:h * F], m2[:, h * F:2 * h * F],
                        Alu.mult)
                tD = sm.tile([P, F], DT, tag="tD")
                nc.vector.tensor_tensor(tD[:], m2[:, 0:F], m2[:, F:2 * F],
                                        Alu.mult)
                nc.vector.tensor_tensor(RS[:, F:2 * F], SPt[:, col], tD[:],
                                        Alu.mult)

            # One Exp op serves tanh(R*1e4), tanh(SP*1e4) and exp(min(R,100)):
            #   cols 0:2F  : t = exp(min(-2e4*x, 87))   [tanh arg, x = R | SP]
            #   cols 2F:3F : e = exp(min(R, 100))
            # tanh(x*1e4) = 2/(1+t) - 1  (exact for saturated x; the arg
            # clamp at 87 keeps t finite so the formula yields -1 there)
            Rv = RS[:, 0:F] if s > 0 else Rst[:, col]
            SPv = RS[:, F:2 * F] if s > 0 else SPt[:, col]
            X3 = sm.tile([P, 3 * F], DT, tag="X3")
            E3 = sm.tile([P, 3 * F], DT, tag="E3")
            if s > 0:
                nc.vector.tensor_scalar(X3[:, 0:2 * F], RS[:], NTANH, 87.0,
                                        Alu.mult, Alu.min)
            else:
                nc.vector.tensor_scalar(X3[:, 0:F], Rv, NTANH, 87.0,
                                        Alu.mult, Alu.min)
                nc.vector.tensor_scalar(X3[:, F:2 * F], SPv, NTANH, 87.0,
                                        Alu.mult, Alu.min)
            nc.vector.tensor_scalar(X3[:, 2 * F:3 * F], Rv, LOG_LIM,
                                    None, Alu.min)
            nc.scalar.activation(E3[:], X3[:], Act.Exp)
            D2 = sm.tile([P, 2 * F], DT, tag="D2")
            TH = sm.tile([P, 2 * F], DT, tag="TH")
            nc.vector.tensor_scalar(D2[:], E3[:, 0:2 * F], 1.0, None, Alu.add)
            nc.vector.reciprocal(D2[:], D2[:])
            nc.vector.tensor_scalar(TH[:], D2[:], 2.0, -1.0, Alu.mult, Alu.add)
            lin = TH[:, 0:F]
            lsig = TH[:, F:2 * F]

            # vs_new = g*lin + c*lsig  (written straight into the history)
            tE = sm.tile([P, F], DT, tag="tE")
            tFt = sm.tile([P, F], DT, tag="tF")
            vsnew = sm.tile([P, F], DT, tag="vsnew")
            vsdst = vsh[:, col] if s < S - 1 else vsnew[:]
            nc.vector.tensor_tensor(tE[:], gs, lin, Alu.mult)
            nc.vector.tensor_tensor(tFt[:], cs, lsig, Alu.mult)
            nc.vector.tensor_tensor(vsdst, tE[:], tFt[:], Alu.add)

            # vm_new = g*|R| + c*e ; |R| = max(R, -R) (abs_max is reduce-only)
            tG = sm.tile([P, F], DT, tag="tG")
            tH = sm.tile([P, F], DT, tag="tH")
            aR = sm.tile([P, F], DT, tag="aR")
            vmnew = sm.tile([P, F], DT, tag="vmnew")
            nc.vector.tensor_scalar(tG[:], Rv, -1.0, None, Alu.mult)
            nc.vector.tensor_tensor(aR[:], Rv, tG[:], Alu.max)
            nc.vector.tensor_tensor(tG[:], gs, aR[:], Alu.mult)
            nc.vector.tensor_tensor(tH[:], cs, E3[:, 2 * F:3 * F], Alu.mult)
            nc.vector.tensor_tensor(vmnew[:], tG[:], tH[:], Alu.add)

            if s == S - 1:
                nc.vector.tensor_tensor(outt[:], vsnew[:], vmnew[:], Alu.mult)
            else:
                nc.vector.tensor_tensor(sgh[:, col], vsdst, vmnew[:],
                                        Alu.mult)
                # lm = ln(clip(vm, 1e-12, FLT_MAX)), but ACT Ln only covers
                # |x| <= 2^64 while vm reaches FLT_MAX. Split off the float
                # exponent with integer bit ops: vm = m * 2^E, m in [1,2):
                #   lm = ln(m) + (E_biased - 127)*ln2
                # +inf vm is reconstructed to lm=+inf via mb; NaN vm positions
                # stay NaN downstream through the sg chain regardless of lm.
                tI = sm.tile([P, F], DT, tag="tI")
                tJ = sm.tile([P, F], DT, tag="tJ")
                tMm = sm.tile([P, F], DT, tag="tMm")
                tEx = sm.tile([P, F], DT, tag="tEx")
                mb = sm.tile([P, F], DT, tag="mb")
                nc.vector.tensor_scalar(tI[:], vmnew[:], 1e-12, FLT_MAX,
                                        Alu.max, Alu.min)
                tExU = sm.tile([P, F], mybir.dt.uint32, tag="tExU")
                tIu = tI[:].bitcast(mybir.dt.uint32)
                nc.vector.tensor_scalar(tExU[:], tIu, 23, None,
                                        Alu.logical_shift_right)
                nc.vector.tensor_scalar(
                    tMm[:].bitcast(mybir.dt.uint32), tIu,
                    0x007FFFFF, 0x3F800000, Alu.bitwise_and, Alu.bitwise_or)
                nc.scalar.activation(tJ[:], tMm[:], Act.Ln)
                # u32 -> f32 conversion fused into the affine (non-bitvec
                # ALU ops cast their input)
                nc.vector.tensor_scalar(tEx[:], tExU[:], LN2, -127.0 * LN2,
                                        Alu.mult, Alu.add)
                nc.vector.tensor_tensor(tJ[:], tJ[:], tEx[:], Alu.add)
                nc.vector.tensor_scalar(mb[:], vmnew[:], FLT_MAX, 0.0,
                                        Alu.subtract, Alu.max)
                nc.vector.tensor_tensor(lmh[:, col], tJ[:], mb[:], Alu.add)

        nc.sync.dma_start(out_d.ap(), outt[:])

    nc.compile()
    return nc


def pack_core(V_mag_b, V_sign_b, O_b, G_b, F):
    """Layout-only host prep for one core. All inputs [Tc,...] with Tc=128*F."""
    Tc = P * F
    vm0 = np.ascontiguousarray(V_mag_b[:, :16].reshape(P, F * 16))
    vs0 = np.ascontiguousarray(V_sign_b[:, :16].reshape(P, F * 16))
    g = np.ascontiguousarray(
        G_b.reshape(P, F, S).transpose(0, 2, 1).reshape(P, S * F))
    ost = np.ascontiguousarray(
        O_b[:, :, :16].reshape(P, F, S, 16).transpose(0, 2, 1, 3)
        .reshape(P, S * F * 16))
    cols = [O_b[:, s, 16:16 + s] for s in range(1, S)]
    tri = np.concatenate(cols, axis=1)  # [Tc, 120]
    ody = np.ascontiguousarray(
        tri.reshape(P, F, TRI).transpose(0, 2, 1).reshape(P, TRI * F))
    return {"vm0": vm0, "vs0": vs0, "g": g, "ost": ost, "ody": ody}


_PROGRAM = None


def _get_program():
    global _PROGRAM
    if _PROGRAM is None:
        _PROGRAM = build_program(T // P)
    return _PROGRAM


def kernel(V_mag, V_sign, O, G):
    V_mag = np.asarray(V_mag, f32)
    V_sign = np.asarray(V_sign, f32)
    O = np.asarray(O, f32)
    G = np.asarray(G, f32)
    F = T // P
    nc = _get_program()
    in_maps = [pack_core(V_mag[b], V_sign[b], O[b], G[b], F) for b in range(B)]
    res = run_bass_kernel_spmd(nc, in_maps, core_ids=list(range(B)))
    out = np.zeros((B, T), f32)
    for b in range(B):
        out[b] = res.results[b]["out"].reshape(T)
    return out
